# revision 1
# baseline (speedup 1.0000x reference)
"""Multi-head self-attention (B=4,S=2048,D=1024,H=16,DH=64, causal) on 8 trn2 cores.

Sharding: core c -> batch b=c//2, head-group g=c%2 (8 heads each).
Per-core: QKV projections (bf16 matmul, f32 accum), S^T = K@Q^T attention with
ones-column denominator trick, exp on ACT (no max-subtraction: |scores|<~25
safe in f32), renorm via reciprocal+partition_broadcast, output projection
producing the partial out^T. Host sums the two head-group partials per batch.

K-projection quirk (reference views k as (B,S,DH,H)): head h uses Wk rows
[dh*16+h for dh in range(64)] -- handled by host-side row gather.
"""
import numpy as np

import concourse.mybir as mybir
import concourse.tile as tile
from concourse import bacc
from concourse.bass_utils import run_bass_kernel_spmd

F32 = mybir.dt.float32
BF16 = mybir.dt.bfloat16
AF = mybir.ActivationFunctionType

B, S, D, H, DH = 4, 2048, 1024, 16, 64
FG = 512          # features per head-group (8 heads * 64)
N_CORES = 8
SCALE = 0.125     # 1/sqrt(64)

_NC = None


def _build():
    nc = bacc.Bacc("TRN2", target_bir_lowering=False, debug=False,
                   num_devices=N_CORES, enable_asserts=False)
    xT_d = nc.dram_tensor("xT", [D, S], F32, kind="ExternalInput").ap()
    wqT_d = nc.dram_tensor("wqT", [D, FG], F32, kind="ExternalInput").ap()
    wkT_d = nc.dram_tensor("wkT", [D, FG], F32, kind="ExternalInput").ap()
    wvT_d = nc.dram_tensor("wvT", [D, FG], F32, kind="ExternalInput").ap()
    wpT_d = nc.dram_tensor("wpT", [FG, D], F32, kind="ExternalInput").ap()
    bqs_d = nc.dram_tensor("bqs", [128, 4], F32, kind="ExternalInput").ap()
    bks_d = nc.dram_tensor("bks", [128, 4], F32, kind="ExternalInput").ap()
    bvs_d = nc.dram_tensor("bvs", [128, 4], F32, kind="ExternalInput").ap()
    bps_d = nc.dram_tensor("bps", [128, 8], F32, kind="ExternalInput").ap()
    msk_d = nc.dram_tensor("msk", [4, 128, 512], F32, kind="ExternalInput").ap()
    out_d = nc.dram_tensor("outT", [D, S], F32, kind="ExternalOutput").ap()

    with tile.TileContext(nc) as tc:
        with tc.tile_pool(name="persist", bufs=1) as pp, \
             tc.tile_pool(name="xin", bufs=3) as xp, \
             tc.tile_pool(name="etile", bufs=8) as ep, \
             tc.tile_pool(name="small", bufs=8) as sp, \
             tc.tile_pool(name="outtile", bufs=4) as op, \
             tc.tile_pool(name="psmm", bufs=4, space="PSUM") as ps_mm, \
             tc.tile_pool(name="psot", bufs=4, space="PSUM") as ps_ot:

            # ---- persistent SBUF tensors ----
            wq = pp.tile([128, 8, FG], BF16)   # [dp, do, f]
            wk = pp.tile([128, 8, FG], BF16)
            wv = pp.tile([128, 8, FG], BF16)
            wp = pp.tile([128, 4, D], BF16)    # [cp, co, j]
            qt = pp.tile([128, 4, S], BF16)    # [fp, fo, s]
            kt = pp.tile([128, 4, S], BF16)
            va = pp.tile([128, 16, 8, DH + 1], BF16)  # [skp, sko, h, dh|1]
            on_ = pp.tile([128, 4, S], BF16)   # renormed out^T  [cp, co, s]
            msk = pp.tile([128, 4, 512], BF16)
            bqs = pp.tile([128, 4], F32)
            bks = pp.tile([128, 4], F32)
            bvs = pp.tile([128, 4], F32)
            bps = pp.tile([128, 8], F32)

            nc.gpsimd.dma_start(wq[:], wqT_d.rearrange("(do dp) f -> dp do f", dp=128))
            nc.gpsimd.dma_start(wk[:], wkT_d.rearrange("(do dp) f -> dp do f", dp=128))
            nc.gpsimd.dma_start(wv[:], wvT_d.rearrange("(do dp) f -> dp do f", dp=128))
            nc.gpsimd.dma_start(wp[:], wpT_d.rearrange("(co cp) j -> cp co j", cp=128))
            nc.gpsimd.dma_start(msk[:], msk_d.rearrange("m p j -> p m j"))
            nc.sync.dma_start(bqs[:], bqs_d[:])
            nc.sync.dma_start(bks[:], bks_d[:])
            nc.sync.dma_start(bvs[:], bvs_d[:])
            nc.sync.dma_start(bps[:], bps_d[:])
            nc.vector.memset(va[:, :, :, DH:DH + 1], 1.0)

            xT_r = xT_d.rearrange("(do dp) s -> dp do s", dp=128)

            # ---- phase B: QKV projections, per 512-wide s block ----
            for sb in range(4):
                xblk = xp.tile([128, 8, 512], BF16)
                nc.gpsimd.dma_start(xblk[:], xT_r[:, :, sb * 512:(sb + 1) * 512])
                # Q^T and K^T: out[f=128, s=512], lhsT = w tile, rhs = x
                for w_sb, dst, bias, scl in ((wq, qt, bqs, SCALE), (wk, kt, bks, 1.0)):
                    for ft in range(4):
                        psq = ps_mm.tile([128, 512], F32, space="PSUM", tag="mm")
                        for do in range(8):
                            nc.tensor.matmul(
                                psq[:], w_sb[:, do, ft * 128:(ft + 1) * 128],
                                xblk[:, do, :],
                                start=(do == 0), stop=(do == 7))
                        nc.scalar.activation(
                            dst[:, ft, sb * 512:(sb + 1) * 512], psq[:],
                            AF.Identity, bias=bias[:, ft:ft + 1], scale=scl)
                # V: out[s=128, f=512], lhsT = x tile, rhs = wv
                for st in range(4):
                    psv = ps_mm.tile([128, 512], F32, space="PSUM", tag="mm")
                    for do in range(8):
                        nc.tensor.matmul(
                            psv[:], xblk[:, do, st * 128:(st + 1) * 128],
                            wv[:, do, :],
                            start=(do == 0), stop=(do == 7))
                    nc.vector.tensor_copy(
                        va[:, sb * 4 + st, :, :DH],
                        psv[:].rearrange("p (h d) -> p h d", h=8))

            # ---- phase C: attention + output projection per 512-wide sq block ----
            def emit_proj(bb, jts):
                # output projection for sq block bb: out^T[j, sq]
                for jt in jts:
                    psj = ps_mm.tile([128, 512], F32, space="PSUM", tag="mm")
                    for co in range(4):
                        nc.tensor.matmul(
                            psj[:], wp[:, co, jt * 128:(jt + 1) * 128],
                            on_[:, co, bb * 512:(bb + 1) * 512],
                            start=(co == 0), stop=(co == 3))
                    ot_sb = op.tile([128, 512], F32, tag="o")
                    nc.scalar.activation(ot_sb[:], psj[:], AF.Identity,
                                         bias=bps[:, jt:jt + 1])
                    nc.sync.dma_start(
                        out_d[jt * 128:(jt + 1) * 128, bb * 512:(bb + 1) * 512],
                        ot_sb[:])

            for b in range(4):
                nt = 4 * b + 4
                for p in range(4):  # head pairs (2p, 2p+1)
                    if b >= 1:  # interleave prev block's projection (dense PE work)
                        emit_proj(b - 1, [2 * p, 2 * p + 1])
                    ot0 = ps_ot.tile([DH + 1, 512], F32, space="PSUM", tag="ot")
                    ot1 = ps_ot.tile([DH + 1, 512], F32, space="PSUM", tag="ot")
                    for t in range(nt):
                        ksl = slice(t * 128, (t + 1) * 128)
                        qsl = slice(b * 512, (b + 1) * 512)
                        s0 = ps_mm.tile([128, 512], F32, space="PSUM", tag="mm")
                        s1 = ps_mm.tile([128, 512], F32, space="PSUM", tag="mm")
                        nc.tensor.matmul(s0[:], kt[0:64, p, ksl], qt[0:64, p, qsl],
                                         start=True, stop=True)
                        nc.tensor.matmul(s1[:], kt[64:128, p, ksl], qt[64:128, p, qsl],
                                         start=True, stop=True)
                        e0 = ep.tile([128, 512], BF16, tag="e")
                        e1 = ep.tile([128, 512], BF16, tag="e")
                        nc.scalar.activation(e0[:], s0[:], AF.Exp)
                        nc.scalar.activation(e1[:], s1[:], AF.Exp)
                        if t >= 4 * b:  # diagonal block: causal mask
                            m = t - 4 * b
                            nc.vector.tensor_tensor(e0[:], e0[:], msk[:, m, :],
                                                    mybir.AluOpType.mult)
                            nc.vector.tensor_tensor(e1[:], e1[:], msk[:, m, :],
                                                    mybir.AluOpType.mult)
                        nc.tensor.matmul(ot0[:], va[:, t, 2 * p, :], e0[:],
                                         start=(t == 0), stop=(t == nt - 1))
                        nc.tensor.matmul(ot1[:], va[:, t, 2 * p + 1, :], e1[:],
                                         start=(t == 0), stop=(t == nt - 1))
                    for h, otp in ((2 * p, ot0), (2 * p + 1, ot1)):
                        rec = sp.tile([1, 512], F32, tag="rec")
                        nc.vector.reciprocal(rec[:], otp[DH:DH + 1, :])
                        rb = sp.tile([DH, 512], F32, tag="rb")
                        nc.gpsimd.partition_broadcast(rb[:], rec[:])
                        r0 = 64 * (h % 2)
                        dst = on_[r0:r0 + 64, h // 2, b * 512:(b + 1) * 512]
                        nc.vector.tensor_tensor(dst, otp[0:DH, :], rb[:],
                                                mybir.AluOpType.mult)
                        nc.vector.tensor_scalar_add(dst, dst,
                                                    bvs[r0:r0 + 64, h // 2:h // 2 + 1])
            emit_proj(3, range(8))

    nc.compile()
    return nc


def kernel(x, Wq, bq, Wk, bk, Wv, bv, Wp, bp):
    global _NC
    if _NC is None:
        _NC = _build()

    x = np.asarray(x, np.float32)
    Wq, bq = np.asarray(Wq, np.float32), np.asarray(bq, np.float32)
    Wk, bk = np.asarray(Wk, np.float32), np.asarray(bk, np.float32)
    Wv, bv = np.asarray(Wv, np.float32), np.asarray(bv, np.float32)
    Wp, bp = np.asarray(Wp, np.float32), np.asarray(bp, np.float32)

    mask = np.zeros((4, 128, 512), np.float32)
    for m in range(4):
        i = np.arange(128)[:, None]
        j = np.arange(512)[None, :]
        mask[m] = (128 * m + i <= j).astype(np.float32)

    in_maps = []
    for c in range(N_CORES):
        b, g = c // 2, c % 2
        hs = range(8 * g, 8 * g + 8)
        kidx = np.array([dh * 16 + h for h in hs for dh in range(DH)])
        fsl = slice(FG * g, FG * (g + 1))
        bp_c = bp if g == 0 else np.zeros_like(bp)
        in_maps.append({
            "xT": np.ascontiguousarray(x[b].T),
            "wqT": np.ascontiguousarray(Wq[fsl].T),
            "wkT": np.ascontiguousarray(Wk[kidx].T),
            "wvT": np.ascontiguousarray(Wv[fsl].T),
            "wpT": np.ascontiguousarray(Wp[:, fsl].T),
            "bqs": np.ascontiguousarray((SCALE * bq[fsl]).reshape(4, 128).T),
            "bks": np.ascontiguousarray(bk[kidx].reshape(4, 128).T),
            "bvs": np.ascontiguousarray(bv[fsl].reshape(4, 128).T),
            "bps": np.ascontiguousarray(bp_c.reshape(8, 128).T),
            "msk": mask,
        })

    res = run_bass_kernel_spmd(_NC, in_maps, core_ids=list(range(N_CORES)))
    out = np.empty((B, S, D), np.float32)
    for b in range(B):
        acc = res.results[2 * b]["outT"] + res.results[2 * b + 1]["outT"]
        out[b] = acc.T
    return out



# revision 5
# speedup vs baseline: 1.4750x; 1.4750x over previous
"""Multi-head self-attention (B=4,S=2048,D=1024,H=16,DH=64, causal) on 8 trn2 cores.

Sharding: core c -> batch b=c//2, head-group g=c%2 (8 heads each).

v2 (from 458us v1 baseline, which was exp/ACT-paced with a 106us reciprocal tax):
 - ScalarE runs ONLY the softmax exps.  Scale folded into host-prescaled Wq;
   bq added for free as the per-partition scalar on the mandatory PSUM->SBUF
   copy; bk dropped exactly (softmax shift invariance: (q+bq)..(k+bk) differs
   from (q+bq)..k by a per-query constant); bv/bp folded into a host-side
   constant (rows of attn sum to 1 => attn@(v+bv) = attn@v + bv, and
   (out+bv)@Wp.T = out@Wp.T + Wp@bv).
 - Exp fused per head-pair: one ACT over [128, 2x512] PSUM (two heads' score
   blocks share a 2-bank PSUM tile), diagonal blocks trimmed to live columns.
 - reciprocal_approx_fast (5x faster than the iterative divider) straight off
   the PSUM denominator row.
 - QKV projections interleaved per 512-wide s-block with the attention of the
   already-available query block, out-proj of block b-1 interleaved into block
   b's head pairs: PE always has independent matmul work (HAM stays warm).
 - Score-pair matmuls adjacent (row groups 0:64 / 64:128 can overlap).

K-projection quirk (reference views k as (B,S,DH,H)): head h uses Wk rows
[dh*16+h for dh in range(64)] -- handled by host-side row gather.
"""
import numpy as np

import concourse.mybir as mybir
import concourse.tile as tile
from concourse import bacc
from concourse.bass_utils import run_bass_kernel_spmd

F32 = mybir.dt.float32
BF16 = mybir.dt.bfloat16
AF = mybir.ActivationFunctionType
MUL = mybir.AluOpType.mult

B, S, D, H, DH = 4, 2048, 1024, 16, 64
FG = 512          # features per head-group (8 heads * 64)
N_CORES = 8
SCALE = 0.125     # 1/sqrt(64)

_NC = None


def _build():
    nc = bacc.Bacc("TRN2", target_bir_lowering=False, debug=False,
                   num_devices=N_CORES, enable_asserts=False)
    xT_d = nc.dram_tensor("xT", [D, S], F32, kind="ExternalInput").ap()
    wqT_d = nc.dram_tensor("wqT", [D, FG], F32, kind="ExternalInput").ap()
    wkT_d = nc.dram_tensor("wkT", [D, FG], F32, kind="ExternalInput").ap()
    wvT_d = nc.dram_tensor("wvT", [D, FG], F32, kind="ExternalInput").ap()
    wpT_d = nc.dram_tensor("wpT", [FG, D], F32, kind="ExternalInput").ap()
    bqs_d = nc.dram_tensor("bqs", [128, 4], F32, kind="ExternalInput").ap()
    msk_d = nc.dram_tensor("msk", [128, 2, 128], F32, kind="ExternalInput").ap()
    out_d = nc.dram_tensor("outT", [D, S], F32, kind="ExternalOutput").ap()

    with tile.TileContext(nc) as tc:
        with tc.tile_pool(name="persist", bufs=1) as pp, \
             tc.tile_pool(name="xin", bufs=2) as xp, \
             tc.tile_pool(name="etile", bufs=4) as ep, \
             tc.tile_pool(name="small", bufs=8) as sp, \
             tc.tile_pool(name="outtile", bufs=3) as op, \
             tc.tile_pool(name="psprs", bufs=2, space="PSUM") as ps_s, \
             tc.tile_pool(name="psot", bufs=2, space="PSUM") as ps_ot, \
             tc.tile_pool(name="psmm", bufs=2, space="PSUM") as ps_mm:

            # ---- persistent SBUF tensors ----
            wq = pp.tile([128, 8, FG], BF16)   # [dp, do, f]
            wk = pp.tile([128, 8, FG], BF16)
            wv = pp.tile([128, 8, FG], BF16)
            wp = pp.tile([128, 4, D], BF16)    # [cp, co, j]
            qt = pp.tile([128, 4, S], BF16)    # [fp, fo, s]
            kt = pp.tile([128, 4, S], BF16)
            va = pp.tile([128, 16, 8, DH + 1], BF16)  # [skp, sko, h, dh|1]
            on_ = pp.tile([128, 4, S], BF16)   # renormed out^T  [cp, co, s]
            msk = pp.tile([128, 2, 128], BF16)
            bqs = pp.tile([128, 4], F32)

            # casting DMAs must go on the gpsimd queue; stagger weight loads so
            # the first matmuls start as early as possible
            nc.gpsimd.dma_start(wq[:], wqT_d.rearrange("(do dp) f -> dp do f", dp=128))
            nc.sync.dma_start(bqs[:], bqs_d[:])
            nc.vector.memset(va[:, :, :, DH:DH + 1], 1.0)

            xT_r = xT_d.rearrange("(do dp) s -> dp do s", dp=128)

            def emit_proj(bb, jts):
                # output projection for s block bb: out^T[j, s] (partial over FG)
                for jt in jts:
                    psj = ps_mm.tile([128, 512], F32, space="PSUM", tag="mm")
                    for co in range(4):
                        nc.tensor.matmul(
                            psj[:], wp[:, co, jt * 128:(jt + 1) * 128],
                            on_[:, co, bb * 512:(bb + 1) * 512],
                            start=(co == 0), stop=(co == 3))
                    osb = op.tile([128, 512], F32, tag="o")
                    nc.vector.tensor_copy(osb[:], psj[:])
                    nc.sync.dma_start(
                        out_d[jt * 128:(jt + 1) * 128, bb * 512:(bb + 1) * 512],
                        osb[:])

            for sb in range(4):
                # ---- QKV projections for s-block sb ----
                xblk = xp.tile([128, 8, 512], BF16, tag="x")
                nc.gpsimd.dma_start(xblk[:], xT_r[:, :, sb * 512:(sb + 1) * 512])
                if sb == 0:
                    nc.gpsimd.dma_start(
                        wk[:], wkT_d.rearrange("(do dp) f -> dp do f", dp=128))
                    nc.gpsimd.dma_start(msk[:], msk_d[:])
                for w_sb, dst, bias in ((wq, qt, bqs), (wk, kt, None)):
                    for ft in range(4):
                        psq = ps_mm.tile([128, 512], F32, space="PSUM", tag="mm")
                        for do in range(8):
                            nc.tensor.matmul(
                                psq[:], w_sb[:, do, ft * 128:(ft + 1) * 128],
                                xblk[:, do, :],
                                start=(do == 0), stop=(do == 7))
                        d = dst[:, ft, sb * 512:(sb + 1) * 512]
                        if bias is not None:
                            nc.vector.tensor_scalar_add(d, psq[:], bias[:, ft:ft + 1])
                        else:
                            nc.vector.tensor_copy(d, psq[:])
                if sb == 0:
                    nc.gpsimd.dma_start(
                        wv[:], wvT_d.rearrange("(do dp) f -> dp do f", dp=128))
                    nc.gpsimd.dma_start(
                        wp[:], wpT_d.rearrange("(co cp) j -> cp co j", cp=128))
                for st in range(4):
                    psv = ps_mm.tile([128, 512], F32, space="PSUM", tag="mm")
                    for do in range(8):
                        nc.tensor.matmul(
                            psv[:], xblk[:, do, st * 128:(st + 1) * 128],
                            wv[:, do, :],
                            start=(do == 0), stop=(do == 7))
                    nc.vector.tensor_copy(
                        va[:, sb * 4 + st, :, :DH],
                        psv[:].rearrange("p (h d) -> p h d", h=8))

                # ---- attention for query block b = sb ----
                b = sb
                nt = 4 * b + 4
                for p in range(4):  # head pairs (2p, 2p+1)
                    if b >= 1:      # interleave prev block's projection
                        emit_proj(b - 1, [2 * p, 2 * p + 1])
                    ot0 = ps_ot.tile([DH + 1, 512], F32, space="PSUM", tag="ot")
                    ot1 = ps_ot.tile([DH + 1, 512], F32, space="PSUM", tag="ot")
                    for t in range(nt):
                        m = t - 4 * b          # >= 0 on diagonal blocks
                        c0 = 128 * m if m > 0 else 0
                        ksl = slice(t * 128, (t + 1) * 128)
                        qsl = slice(b * 512 + c0, (b + 1) * 512)
                        spr = ps_s.tile([128, 2, 512], F32, space="PSUM", tag="s")
                        nc.tensor.matmul(spr[:, 0, c0:], kt[0:64, p, ksl],
                                         qt[0:64, p, qsl], start=True, stop=True)
                        nc.tensor.matmul(spr[:, 1, c0:], kt[64:128, p, ksl],
                                         qt[64:128, p, qsl], start=True, stop=True)
                        e = ep.tile([128, 2, 512], BF16, tag="e")
                        nc.scalar.activation(e[:, :, c0:], spr[:, :, c0:], AF.Exp)
                        if m >= 0:  # diagonal block: causal mask on the 128-strip
                            nc.vector.tensor_tensor(e[:, :, c0:c0 + 128],
                                                    e[:, :, c0:c0 + 128],
                                                    msk[:], MUL)
                        nc.tensor.matmul(ot0[:, c0:], va[:, t, 2 * p, :],
                                         e[:, 0, c0:],
                                         start=(t == 0), stop=(t == nt - 1),
                                         skip_group_check=True)
                        nc.tensor.matmul(ot1[:, c0:], va[:, t, 2 * p + 1, :],
                                         e[:, 1, c0:],
                                         start=(t == 0), stop=(t == nt - 1),
                                         skip_group_check=True)
                    for h, otp in ((2 * p, ot0), (2 * p + 1, ot1)):
                        den = sp.tile([1, 512], F32, tag="den")
                        nc.vector.tensor_copy(den[:], otp[DH:DH + 1, :])
                        rec = sp.tile([1, 512], F32, tag="rec")
                        nc.vector.reciprocal_approx_fast(rec[:], den[:])
                        rb = sp.tile([DH, 512], F32, tag="rb")
                        nc.gpsimd.partition_broadcast(rb[:], rec[:])
                        r0 = 64 * (h % 2)
                        dst = on_[r0:r0 + 64, p, b * 512:(b + 1) * 512]
                        nc.vector.tensor_tensor(dst, otp[0:DH, :], rb[:], MUL)
            emit_proj(3, range(8))

    nc.compile()
    return nc


def kernel(x, Wq, bq, Wk, bk, Wv, bv, Wp, bp):
    global _NC
    if _NC is None:
        _NC = _build()

    x = np.asarray(x, np.float32)
    Wq, bq = np.asarray(Wq, np.float32), np.asarray(bq, np.float32)
    Wk, bk = np.asarray(Wk, np.float32), np.asarray(bk, np.float32)
    Wv, bv = np.asarray(Wv, np.float32), np.asarray(bv, np.float32)
    Wp, bp = np.asarray(Wp, np.float32), np.asarray(bp, np.float32)

    # diagonal-strip causal mask, duplicated for the two heads of a pair
    i = np.arange(128)[:, None]
    j = np.arange(128)[None, :]
    mstrip = (i <= j).astype(np.float32)            # [128, 128]
    msk = np.broadcast_to(mstrip[:, None, :], (128, 2, 128)).copy()

    # host-folded constant: attn rows sum to 1 -> out += bv, then @Wp.T
    host_bias = Wp @ bv + bp                        # [D]

    in_maps = []
    for c in range(N_CORES):
        b, g = c // 2, c % 2
        hs = range(8 * g, 8 * g + 8)
        kidx = np.array([dh * 16 + h for h in hs for dh in range(DH)])
        fsl = slice(FG * g, FG * (g + 1))
        in_maps.append({
            "xT": np.ascontiguousarray(x[b].T),
            "wqT": np.ascontiguousarray((SCALE * Wq[fsl]).T),
            "wkT": np.ascontiguousarray(Wk[kidx].T),
            "wvT": np.ascontiguousarray(Wv[fsl].T),
            "wpT": np.ascontiguousarray(Wp[:, fsl].T),
            "bqs": np.ascontiguousarray((SCALE * bq[fsl]).reshape(4, 128).T),
            "msk": msk,
        })

    res = run_bass_kernel_spmd(_NC, in_maps, core_ids=list(range(N_CORES)))
    out = np.empty((B, S, D), np.float32)
    for b in range(B):
        acc = res.results[2 * b]["outT"] + res.results[2 * b + 1]["outT"]
        out[b] = acc.T + host_bias
    return out
